# revision 17
# baseline (speedup 1.0000x reference)
"""BertEmbedding (scalar-mix + ragged mean-pool + projection) on 8 TRN2 cores.

Full-input contract: kernel(**inputs) takes the unsharded numpy inputs and
returns the full [32, 256, 400] f32 output. Internally: data-parallel over
batch (4 examples per core), proj_w replicated. The host only shards,
casts dtypes (f32 -> bf16) and relayouts (plus uploads data-independent
index/selector constants); all math from inputs to outputs runs on-device.

Key structure (evolved from an f32r baseline through trace-driven rounds):
  - hidden states are uploaded as bf16 (tolerance is 2e-2; bf16 costs ~4e-3)
    and only the first PMAX=288 subword positions per example are uploaded:
    with bert_lens < 3 the per-example total is ~256+-13 (max 279 over the
    batch), so positions past the cumsum total never belong to any word
    (their membership row is all zero). A fallback variant with more
    positions compiles on demand if an input ever exceeds the prefix.
  - positions 0..255 (two 128-chunks): the 4-layer scalar mix runs on the
    Vector engine as a running sum of tensor_scalar (4x DVE mode) +
    tensor_tensor add (2x mode) in layer-arrival order; the 1x-only
    scalar_tensor_tensor op is avoided for the bulk data.
  - positions 256..287 ride as a stacked chunk [4 layers x 32 positions =
    128 partitions] whose pooling matmul contracts layer and position
    jointly against a w-scaled membership matrix - no DVE mix at all.
  - all hidden loads go over HWDGE (sync queue, ~0.6us latency, FIFO) so
    example 0 lands early and per-example compute pipelines; projT/lens
    side loads ride the scalar HWDGE ring; nothing waits on Q7/SWDGE.
  - all matmuls run in bf16 (1 col/cycle; f32r 128-col matmuls run at 1/4
    rate at full clock). A short burst of dummy warm-up matmuls after the
    broadcast matmuls keeps the PE busy through the DMA fill so its DVFS
    p-state (0.65/1.2/2.4 GHz) is ramped when real pooling starts.

Math per example:
  w        = softmax(mix_weights) * gamma                       (ACT/DVE)
  ends     = cumsum(lens); starts = ends - lens                 (DVE scan)
  M[p, j]  = (starts[j] < p+1) & (ends[j] >= p+1)               (DVE, 0/1 bf16)
  mixed    = sum_l w[l] * hid[l]          (positions < 256)     (DVE, bf16)
  pooledT[h, j] = sum_p mixed[p, h] M[p, j]
                + sum_{l,q} hid[l, 256+q, h] (w[l] M[256+q, j]) (PE, bf16)
  out[j, o] = (pooledT[:, j] . projT[:, o]) / max(lens[j], 1)   (PE, bf16;
              the 1/cnt is a per-partition ACT scale on the PSUM copy)

Membership builds are windowed: a chunk starting at position P0 only
reaches words j >= P0//2 (lens <= 2), so chunk c1 builds/pools j in
[64, 256) and the stacked tail chunk j in [128, 256).
"""

import numpy as np

NL, B, SW, H = 4, 32, 512, 768
SL, NOUT = 256, 400
NCORES = 8
BPC = B // NCORES  # examples per core
HC = H // 128      # hidden chunks
JC = SL // 128     # word chunks
PMAX_DEFAULT = 288
N_WARMUP = 20      # PE p-state warm-up matmuls

_NC_CACHE = {}
LAST_RESULT = None  # BassKernelResults of the last run (for profiling)


def _build_nc(pmax):
    import concourse.bacc as bacc
    import concourse.tile as tile
    from concourse import mybir

    f32 = mybir.dt.float32
    f32r = mybir.dt.float32r
    bf16 = mybir.dt.bfloat16
    i32 = mybir.dt.int32
    Alu = mybir.AluOpType
    Act = mybir.ActivationFunctionType
    Axis = mybir.AxisListType

    PF = pmax // 128          # full 128-position chunks (pre-mixed on DVE)
    C2 = pmax - 128 * PF      # stacked tail positions (32 here; 4l x 32q = 128)
    assert C2 in (0, 32)

    nc = bacc.Bacc(None)
    hidm = nc.dram_tensor("hidm", [BPC, 128, NL, PF * H], bf16, kind="ExternalInput")
    if C2:
        # stacked tail: partition 32l + q holds layer l of position 128*PF + q
        hidc = nc.dram_tensor("hidc", [BPC, NL * C2, H], bf16, kind="ExternalInput")
        sel32 = nc.dram_tensor("sel32", [NL, NL * C2], f32, kind="ExternalInput")
    lens = nc.dram_tensor("lens", [BPC, SL], i32, kind="ExternalInput")
    mw = nc.dram_tensor("mw", [1, NL], f32, kind="ExternalInput")
    gam = nc.dram_tensor("gam", [1, 1], f32, kind="ExternalInput")
    projT = nc.dram_tensor("projT", [128, HC * NOUT], bf16, kind="ExternalInput")
    sel = nc.dram_tensor("sel", [BPC, BPC * 128], f32, kind="ExternalInput")
    # csall[p, c] = 1 + position of (partition p, chunk c); last col is the
    # stacked tail (257 + p%32). Pure index math, uploaded as f32.
    csall = nc.dram_tensor("csall", [128, PF + (1 if C2 else 0)], f32, kind="ExternalInput")
    out = nc.dram_tensor("out", [BPC, SL, NOUT], f32, kind="ExternalOutput")

    with tile.TileContext(nc) as tc:
        with (
            tc.tile_pool(name="const", bufs=1) as const,
            tc.tile_pool(name="small", bufs=1) as small,
            tc.tile_pool(name="h", bufs=BPC) as hpool,
            tc.tile_pool(name="mx", bufs=2) as mxpool,
            tc.tile_pool(name="mtmp", bufs=2) as mpool,
            tc.tile_pool(name="Mm", bufs=BPC) as Mpool,
            tc.tile_pool(name="se", bufs=BPC) as sepool,
            tc.tile_pool(name="pt", bufs=2) as ptpool,
            tc.tile_pool(name="osb", bufs=2) as opool,
            tc.tile_pool(name="psb", bufs=1, space="PSUM") as ps_b,
            tc.tile_pool(name="psp", bufs=1, space="PSUM") as ps_p,
            tc.tile_pool(name="pso", bufs=1, space="PSUM") as ps_o,
        ):
            # ---- tiny + side loads on the scalar HWDGE ring; the sync ring is
            # reserved exclusively for the hidden-state FIFO so example 0
            # starts landing at ~2.5us ----
            ones_f1 = const.tile([1, 128], f32)
            nc.vector.memset(ones_f1[:], 1.0)
            ones_b = const.tile([1, 512], bf16)
            nc.vector.memset(ones_b[:], 1.0)
            sel_f = const.tile([BPC, BPC * 128], f32)
            nc.scalar.dma_start(sel_f[:], sel[:])
            sel_sb = const.tile([BPC, BPC * 128], f32r)
            nc.vector.tensor_copy(sel_sb[:], sel_f[:])
            if C2:
                sel32_sb = const.tile([NL, NL * C2], f32)
                nc.scalar.dma_start(sel32_sb[:], sel32[:])
            cs_sb = const.tile([128, PF + (1 if C2 else 0)], f32)
            nc.scalar.dma_start(cs_sb[:], csall[:])
            lens_i = small.tile([BPC, SL], i32)
            nc.scalar.dma_start(lens_i[:], lens[:])
            mw_sb = small.tile([1, NL], f32)
            nc.scalar.dma_start(mw_sb[:], mw[:])
            gam_sb = small.tile([1, 1], f32)
            nc.scalar.dma_start(gam_sb[:], gam[:])
            projT_sb = const.tile([128, HC, NOUT], bf16)
            nc.scalar.dma_start(projT_sb[:], projT.rearrange("p (i o) -> p i o", i=HC))
            lensc_i = small.tile([128, JC, BPC], i32)
            for jh in range(JC):
                nc.scalar.dma_start(lensc_i[:, jh, :], lens[:, jh * 128:(jh + 1) * 128].rearrange("b p -> p b"))

            # ---- lens: ends/starts rows (f32r) ----
            lensf = small.tile([BPC, SL], f32)
            nc.vector.tensor_copy(lensf[:], lens_i[:])
            ends_r = small.tile([BPC, SL], f32r)
            nc.vector.tensor_tensor_scan(out=ends_r[:], data0=lensf[:], data1=lensf[:], initial=0.0, op0=Alu.add, op1=Alu.bypass)
            starts_r = small.tile([BPC, SL], f32r)
            nc.vector.tensor_sub(starts_r[:], ends_r[:], lensf[:])

            # ---- 1/cnt per word (j on partitions) ----
            lensc_f = small.tile([128, JC, BPC], f32)
            nc.vector.tensor_copy(lensc_f[:], lensc_i[:])
            lensc_m = small.tile([128, JC, BPC], f32)
            nc.vector.tensor_scalar_max(lensc_m[:], lensc_f[:], 1.0)
            invcnt = small.tile([128, JC, BPC], f32)
            nc.vector.reciprocal(out=invcnt[:], in_=lensc_m[:])

            # ---- softmax(mix_weights) * gamma ----
            mmax = small.tile([1, 1], f32)
            nc.vector.tensor_reduce(out=mmax[:], in_=mw_sb[:], axis=Axis.X, op=Alu.max)
            nmax = small.tile([1, 1], f32)
            nc.vector.tensor_scalar(out=nmax[:], in0=mmax[:], scalar1=-1.0, scalar2=None, op0=Alu.mult)
            mexp = small.tile([1, NL], f32)
            nc.scalar.activation(out=mexp[:], in_=mw_sb[:], func=Act.Exp, bias=nmax[:], scale=1.0)
            msum = small.tile([1, 1], f32)
            nc.vector.tensor_reduce(out=msum[:], in_=mexp[:], axis=Axis.X, op=Alu.add)
            mrec = small.tile([1, 1], f32)
            nc.vector.reciprocal(out=mrec[:], in_=msum[:])
            w_row = small.tile([1, NL], f32)
            nc.vector.tensor_scalar(out=w_row[:], in0=mexp[:], scalar1=mrec[:], scalar2=gam_sb[:], op0=Alu.mult, op1=Alu.mult)

            # ---- PE front block: all broadcast matmuls, then warm-up ----
            # w broadcast to [128, NL]
            ps_w = ps_o.tile([128, NL], f32, tag="po")
            nc.tensor.matmul(out=ps_w[:], lhsT=ones_f1[:], rhs=w_row[:], start=True, stop=True)
            w_sb = small.tile([128, NL], f32)
            nc.scalar.copy(w_sb[:], ps_w[:])
            if C2:
                # w as a column, then stacked per-partition w (w[p // 32])
                ps_wt = ps_b.tile([NL, 1], f32, tag="se", name="ps_wt")
                nc.tensor.matmul(out=ps_wt[:], lhsT=w_row[:], rhs=ones_f1[:, 0:1], start=True, stop=True)
                wt_sb = small.tile([NL, 1], f32)
                nc.scalar.copy(wt_sb[:], ps_wt[:])
                ps_ws = ps_b.tile([NL * C2, 1], f32, tag="se", name="ps_ws")
                nc.tensor.matmul(out=ps_ws[:], lhsT=sel32_sb[:], rhs=wt_sb[:], start=True, stop=True)
                wst_sb = small.tile([NL * C2, 1], f32)
                nc.scalar.copy(wst_sb[:], ps_ws[:])

            # starts/ends broadcast per example: [128, 0:SL]=starts, [SL:2SL]=ends
            ses = []
            for b in range(BPC):
                ps_se = ps_b.tile([128, 2 * SL], f32, tag="se")
                sel_b = sel_sb[:, b * 128:(b + 1) * 128]
                nc.tensor.matmul(out=ps_se[:, 0:SL], lhsT=sel_b, rhs=starts_r[:], start=True, stop=True)
                nc.tensor.matmul(out=ps_se[:, SL:2 * SL], lhsT=sel_b, rhs=ends_r[:], start=True, stop=True)
                se_sb = sepool.tile([128, 2 * SL], f32, tag="sesb")
                nc.scalar.copy(se_sb[:], ps_se[:])
                ses.append(se_sb)

            # p-state warm-up: keeps the PE clock ramped through the DMA fill
            warm = ps_o.tile([128, NOUT], f32, tag="po", name="warm")
            for _ in range(N_WARMUP):
                nc.tensor.matmul(out=warm[:], lhsT=ones_b[:, 0:128], rhs=ones_b[:, 0:NOUT], start=True, stop=True)

            # ---- per-example membership (DVE, windowed, just-in-time;
            # Pool/GPSIMD lacks comparison opcodes so this stays on DVE) ----
            def emit_memb(b):
                se_sb = ses[b]
                Mt = Mpool.tile([128, PF, SL], bf16, tag="M")
                for c in range(PF):
                    j0 = 64 * c
                    w = SL - j0
                    csc = cs_sb[:, c:c + 1]
                    m2 = mpool.tile([128, SL], f32, tag="m2")
                    nc.vector.tensor_scalar(
                        out=m2[:, :w], in0=se_sb[:, SL + j0:2 * SL], scalar1=csc,
                        scalar2=None, op0=Alu.is_ge)
                    nc.vector.scalar_tensor_tensor(
                        out=Mt[:, c, j0:], in0=se_sb[:, j0:SL], scalar=csc,
                        in1=m2[:, :w], op0=Alu.is_lt, op1=Alu.mult)
                Mc = None
                if C2:
                    # stacked tail membership with w folded in: rows (l, q)
                    j0 = 64 * PF
                    w = SL - j0
                    csc = cs_sb[:, PF:PF + 1]
                    Mc = Mpool.tile([128, SL - 64 * PF], bf16, tag="Mc", name="Mc")
                    m2 = mpool.tile([128, SL], f32, tag="m2")
                    nc.vector.tensor_scalar(
                        out=m2[:, :w], in0=se_sb[:, SL + j0:2 * SL], scalar1=csc,
                        scalar2=wst_sb[:], op0=Alu.is_ge, op1=Alu.mult)
                    nc.vector.scalar_tensor_tensor(
                        out=Mc[:], in0=se_sb[:, j0:SL], scalar=csc,
                        in1=m2[:, :w], op0=Alu.is_lt, op1=Alu.mult)
                return Mt, Mc

            # ---- per-example hidden loads (HWDGE sync FIFO) ----
            def emit_loads(b):
                ht = hpool.tile([128, NL, PF * H], bf16, tag="hm")
                ht2 = None
                if C2:
                    ht2 = hpool.tile([NL * C2, H], bf16, tag="hc", name="ht2")
                    nc.sync.dma_start(ht2[:], hidc[b, :, :])
                for l in range(NL):
                    nc.sync.dma_start(ht[:, l, :], hidm[b, :, l, :])
                return ht, ht2

            # ---- 4-layer running-sum mix in layer-arrival order (DVE).
            # The last add is split into chunk halves so the c0 pooling
            # matmuls can start while the c1 half is still mixing. ----
            def emit_mix(b, ht):
                mxm = mxpool.tile([128, PF * H], bf16, tag="mxm")
                tmp = mxpool.tile([128, PF * H], bf16, tag="tmp")
                nc.vector.tensor_scalar(out=mxm[:], in0=ht[:, 0, :], scalar1=w_sb[:, 0:1], scalar2=None, op0=Alu.mult)
                for l in range(1, NL):
                    nc.vector.tensor_scalar(out=tmp[:], in0=ht[:, l, :], scalar1=w_sb[:, l:l + 1], scalar2=None, op0=Alu.mult)
                    if l < NL - 1:
                        nc.vector.tensor_add(mxm[:], mxm[:], tmp[:])
                    else:
                        for c in range(PF):
                            s = slice(c * H, (c + 1) * H)
                            nc.vector.tensor_add(mxm[:, s], mxm[:, s], tmp[:, s])
                return mxm

            def emit_pool(b, mxm, ht2, Mt, Mc):
                # pooledT[h, j] = sum_p mixed[p, h] M[p, j]; the stacked tail
                # chunk contracts (layer, position) jointly with w in Mc.
                # c-outer order: all banks consume chunk c before chunk c+1,
                # so pooling starts as soon as the c0 half of the mix lands.
                ptsb = ptpool.tile([128, HC, SL], bf16, tag="pt")
                pps = [ps_p.tile([128, SL], f32, tag=f"pp{i}", name=f"pp{i}") for i in range(HC)]
                for c in range(PF):
                    j0 = 64 * c
                    for i in range(HC):
                        nc.tensor.matmul(
                            out=pps[i][:, j0:],
                            lhsT=mxm[:, c * H + i * 128: c * H + (i + 1) * 128],
                            rhs=Mt[:, c, j0:],
                            start=(c == 0),
                            stop=(c == PF - 1 and not C2),
                            skip_group_check=True,
                        )
                for i in range(HC):
                    if C2:
                        nc.tensor.matmul(
                            out=pps[i][:, 64 * PF:],
                            lhsT=ht2[:, i * 128:(i + 1) * 128],
                            rhs=Mc[:],
                            start=False,
                            stop=True,
                            skip_group_check=True,
                        )
                    nc.scalar.copy(ptsb[:, i, :], pps[i][:])
                return ptsb

            def emit_proj(b, ptsb):
                for jh in range(JC):
                    po = ps_o.tile([128, NOUT], f32, tag="po")
                    for i in range(HC):
                        nc.tensor.matmul(
                            out=po[:],
                            lhsT=ptsb[:, i, jh * 128:(jh + 1) * 128],
                            rhs=projT_sb[:, i, :],
                            start=(i == 0),
                            stop=(i == HC - 1),
                        )
                    osb = opool.tile([128, NOUT], f32, tag="o")
                    nc.scalar.activation(out=osb[:], in_=po[:], func=Act.Copy, scale=invcnt[:, jh, b:b + 1])
                    nc.scalar.dma_start(out[b, jh * 128:(jh + 1) * 128, :], osb[:])

            loads = [emit_loads(b) for b in range(BPC)]
            prev = None  # (b, ptsb) pending projection
            for b in range(BPC):
                ht, ht2 = loads[b]
                Mt, Mc = emit_memb(b)
                mxm = emit_mix(b, ht)
                ptsb = emit_pool(b, mxm, ht2, Mt, Mc)
                if prev is not None:
                    emit_proj(*prev)
                prev = (b, ptsb)
            emit_proj(*prev)

    nc.finalize()
    return nc


def _get_nc(pmax):
    if pmax not in _NC_CACHE:
        _NC_CACHE[pmax] = _build_nc(pmax)
    return _NC_CACHE[pmax]


def kernel(subwords=None, bert_lens=None, bert_mask=None, hidden_states=None,
           mix_weights=None, gamma=None, proj_w=None, **_ignored):
    global LAST_RESULT
    import os
    import ml_dtypes
    from concourse.bass_utils import run_bass_kernel_spmd

    bf16 = ml_dtypes.bfloat16
    lens_np = np.asarray(bert_lens).astype(np.int32)

    # pick the smallest compiled position-prefix that covers every example
    need = int(lens_np.sum(axis=1).max())
    pmax = PMAX_DEFAULT
    if need > pmax:
        pmax = 384 if need <= 384 else 512
    nc = _get_nc(pmax)
    PF = pmax // 128
    C2 = pmax - 128 * PF
    CH = PF + (1 if C2 else 0)

    hs = np.asarray(hidden_states, dtype=np.float32).astype(bf16)  # [NL,B,SW,H]
    mw_np = np.asarray(mix_weights, dtype=np.float32).reshape(1, NL)
    gam_np = np.asarray(gamma, dtype=np.float32).reshape(1, 1)
    # projT[p, i*NOUT + o] = proj_w[o, 128*i + p]
    projT_np = np.ascontiguousarray(
        np.asarray(proj_w, dtype=np.float32).astype(bf16).T
        .reshape(HC, 128, NOUT).transpose(1, 0, 2).reshape(128, HC * NOUT))
    sel_np = np.zeros((BPC, BPC * 128), dtype=np.float32)
    for b in range(BPC):
        sel_np[b, b * 128:(b + 1) * 128] = 1.0
    cs_np = np.empty((128, CH), dtype=np.float32)
    p = np.arange(128)
    for c in range(PF):
        cs_np[:, c] = 1 + 128 * c + p
    if C2:
        cs_np[:, PF] = 1 + 128 * PF + (p % C2)
        sel32_np = np.zeros((NL, NL * C2), dtype=np.float32)
        for l in range(NL):
            sel32_np[l, l * C2:(l + 1) * C2] = 1.0

    in_maps = []
    for cidx in range(NCORES):
        sl = slice(cidx * BPC, (cidx + 1) * BPC)
        hsb = hs[:, sl]  # [NL, BPC, SW, H]
        # hidm[b, part, l, c*H + h] = hs[l, b, 128c + part, h]
        hidm = np.ascontiguousarray(
            hsb[:, :, :128 * PF].reshape(NL, BPC, PF, 128, H)
            .transpose(1, 3, 0, 2, 4).reshape(BPC, 128, NL, PF * H))
        m = {
            "hidm": hidm,
            "lens": np.ascontiguousarray(lens_np[sl]),
            "mw": mw_np,
            "gam": gam_np,
            "projT": projT_np,
            "sel": sel_np,
            "csall": cs_np,
        }
        if C2:
            # stacked tail: partition 32l + q = layer l, position 128*PF + q
            m["hidc"] = np.ascontiguousarray(
                hsb[:, :, 128 * PF:128 * PF + C2].transpose(1, 0, 2, 3)
                .reshape(BPC, NL * C2, H))
            m["sel32"] = sel32_np
        in_maps.append(m)

    trace = bool(int(os.environ.get("KERNEL_TRACE", "0")))
    LAST_RESULT = run_bass_kernel_spmd(nc, in_maps, list(range(NCORES)), trace=trace)
    res = LAST_RESULT.results
    return np.concatenate([np.asarray(r["out"], dtype=np.float32) for r in res], axis=0)


# revision 23
# speedup vs baseline: 1.2510x; 1.2510x over previous
"""BertEmbedding (scalar-mix + ragged mean-pool + projection) on 8 TRN2 cores.

Full-input contract: kernel(**inputs) takes the unsharded numpy inputs and
returns the full [32, 256, 400] f32 output. Internally: data-parallel over
batch (4 examples per core), proj_w replicated. The host only shards,
casts dtypes (f32 -> bf16) and relayouts (plus uploads data-independent
index/selector constants); all math from inputs to outputs runs on-device.

Key structure (evolved from an f32r baseline through trace-driven rounds):
  - hidden states are uploaded as bf16 (tolerance is 2e-2; bf16 costs ~4e-3)
    and only the first PMAX=288 subword positions per example are uploaded:
    with bert_lens < 3 the per-example total is ~256+-13 (max 279 over the
    batch), so positions past the cumsum total never belong to any word
    (their membership row is all zero). A fallback variant with more
    positions compiles on demand if an input ever exceeds the prefix.
  - positions 0..255 (two 128-chunks): the 4-layer scalar mix runs on the
    Vector engine as a running sum of tensor_scalar (4x DVE mode) +
    tensor_tensor add (2x mode) in layer-arrival order; the 1x-only
    scalar_tensor_tensor op is avoided for the bulk data.
  - positions 256..287 ride as a stacked chunk [4 layers x 32 positions =
    128 partitions] whose pooling matmul contracts layer and position
    jointly against a w-scaled membership matrix - no DVE mix at all.
  - all hidden loads go over HWDGE (sync queue, ~0.6us latency, FIFO) so
    example 0 lands early and per-example compute pipelines; projT/lens
    side loads ride the scalar HWDGE ring; nothing waits on Q7/SWDGE.
  - all matmuls run in bf16 (1 col/cycle; f32r 128-col matmuls run at 1/4
    rate at full clock). A short burst of dummy warm-up matmuls after the
    broadcast matmuls keeps the PE busy through the DMA fill so its DVFS
    p-state (0.65/1.2/2.4 GHz) is ramped when real pooling starts.

Math per example:
  w        = softmax(mix_weights) * gamma                       (ACT/DVE)
  ends     = cumsum(lens); starts = ends - lens                 (DVE scan)
  M[p, j]  = (starts[j] < p+1) & (ends[j] >= p+1)               (DVE, 0/1 bf16)
  mixed    = sum_l w[l] * hid[l]          (positions < 256)     (DVE, bf16)
  pooledT[h, j] = sum_p mixed[p, h] M[p, j]
                + sum_{l,q} hid[l, 256+q, h] (w[l] M[256+q, j]) (PE, bf16)
  out[j, o] = (pooledT[:, j] . projT[:, o]) / max(lens[j], 1)   (PE, bf16;
              the 1/cnt is a per-partition ACT scale on the PSUM copy)

Membership builds are windowed: a chunk starting at position P0 only
reaches words j >= P0//2 (lens <= 2), so chunk c1 builds/pools j in
[64, 256) and the stacked tail chunk j in [128, 256).
"""

import numpy as np

NL, B, SW, H = 4, 32, 512, 768
SL, NOUT = 256, 400
NCORES = 8
BPC = B // NCORES  # examples per core
HC = H // 128      # hidden chunks
JC = SL // 128     # word chunks
PMAX_DEFAULT = 288
N_WARMUP = 16      # PE p-state warm-up matmuls

_NC_CACHE = {}
LAST_RESULT = None  # BassKernelResults of the last run (for profiling)


def _build_nc(pmax):
    import concourse.bacc as bacc
    import concourse.tile as tile
    from concourse import mybir

    f32 = mybir.dt.float32
    f32r = mybir.dt.float32r
    bf16 = mybir.dt.bfloat16
    i32 = mybir.dt.int32
    Alu = mybir.AluOpType
    Act = mybir.ActivationFunctionType
    Axis = mybir.AxisListType

    PF = pmax // 128          # full 128-position chunks (pre-mixed on DVE)
    C2 = pmax - 128 * PF      # stacked tail positions (32 here; 4l x 32q = 128)
    assert C2 in (0, 32)

    nc = bacc.Bacc(None)
    hidm = nc.dram_tensor("hidm", [BPC, 128, NL, PF * H], bf16, kind="ExternalInput")
    if C2:
        # stacked tail: partition 32l + q holds layer l of position 128*PF + q
        hidc = nc.dram_tensor("hidc", [BPC, NL * C2, H], bf16, kind="ExternalInput")
        sel32 = nc.dram_tensor("sel32", [NL, NL * C2], f32, kind="ExternalInput")
    lens = nc.dram_tensor("lens", [BPC, SL], i32, kind="ExternalInput")
    mw = nc.dram_tensor("mw", [1, NL], f32, kind="ExternalInput")
    gam = nc.dram_tensor("gam", [1, 1], f32, kind="ExternalInput")
    projT = nc.dram_tensor("projT", [128, HC * NOUT], bf16, kind="ExternalInput")
    sel = nc.dram_tensor("sel", [BPC, BPC * 128], f32, kind="ExternalInput")
    # csall[p, c] = 1 + position of (partition p, chunk c); last col is the
    # stacked tail (257 + p%32). Pure index math, uploaded as f32.
    csall = nc.dram_tensor("csall", [128, PF + (1 if C2 else 0)], f32, kind="ExternalInput")
    out = nc.dram_tensor("out", [BPC, SL, NOUT], f32, kind="ExternalOutput")

    with tile.TileContext(nc) as tc:
        with (
            tc.tile_pool(name="const", bufs=1) as const,
            tc.tile_pool(name="small", bufs=1) as small,
            tc.tile_pool(name="h", bufs=BPC) as hpool,
            tc.tile_pool(name="mx", bufs=2) as mxpool,
            tc.tile_pool(name="mtmp", bufs=2) as mpool,
            tc.tile_pool(name="Mm", bufs=2) as Mpool,
            tc.tile_pool(name="se", bufs=2) as sepool,
            tc.tile_pool(name="pt", bufs=2) as ptpool,
            tc.tile_pool(name="osb", bufs=2) as opool,
            tc.tile_pool(name="psb", bufs=1, space="PSUM") as ps_b,
            tc.tile_pool(name="psp", bufs=1, space="PSUM") as ps_p,
            tc.tile_pool(name="pso", bufs=1, space="PSUM") as ps_o,
        ):
            # ---- tiny input loads on the sync ring just ahead of the hidden
            # FIFO (simple descriptors, cheap descgen). The descriptor-heavy
            # projT/lensc loads go through the otherwise-idle GPSIMD SWDGE so
            # their descgen never blocks an engine with real work: descgen
            # runs on the issuing sequencer, which starved the scalar engine
            # (softmax/copies) in earlier rounds. ----
            ones_f1 = const.tile([1, 128], f32)
            nc.vector.memset(ones_f1[:], 1.0)
            ones_b = const.tile([1, 512], bf16)
            nc.vector.memset(ones_b[:], 1.0)
            sel_f = const.tile([BPC, BPC * 128], f32)
            nc.sync.dma_start(sel_f[:], sel[:])
            sel_sb = const.tile([BPC, BPC * 128], f32r)
            nc.vector.tensor_copy(sel_sb[:], sel_f[:])
            if C2:
                sel32_sb = const.tile([NL, NL * C2], f32)
                nc.sync.dma_start(sel32_sb[:], sel32[:])
            cs_sb = const.tile([128, PF + (1 if C2 else 0)], f32)
            nc.sync.dma_start(cs_sb[:], csall[:])
            lens_i = small.tile([BPC, SL], i32)
            nc.sync.dma_start(lens_i[:], lens[:])
            mw_sb = small.tile([1, NL], f32)
            nc.sync.dma_start(mw_sb[:], mw[:])
            gam_sb = small.tile([1, 1], f32)
            nc.sync.dma_start(gam_sb[:], gam[:])
            projT_sb = const.tile([128, HC, NOUT], bf16)
            nc.gpsimd.dma_start(projT_sb[:], projT.rearrange("p (i o) -> p i o", i=HC))
            lensc_i = small.tile([128, JC, BPC], i32)
            for jh in range(JC):
                nc.gpsimd.dma_start(lensc_i[:, jh, :], lens[:, jh * 128:(jh + 1) * 128].rearrange("b p -> p b"))

            # ---- lens: ends/starts rows (f32r) ----
            lensf = small.tile([BPC, SL], f32)
            nc.vector.tensor_copy(lensf[:], lens_i[:])
            ends_r = small.tile([BPC, SL], f32r)
            nc.vector.tensor_tensor_scan(out=ends_r[:], data0=lensf[:], data1=lensf[:], initial=0.0, op0=Alu.add, op1=Alu.bypass)
            starts_r = small.tile([BPC, SL], f32r)
            nc.vector.tensor_sub(starts_r[:], ends_r[:], lensf[:])

            # ---- 1/cnt per word (j on partitions) ----
            lensc_f = small.tile([128, JC, BPC], f32)
            nc.vector.tensor_copy(lensc_f[:], lensc_i[:])
            lensc_m = small.tile([128, JC, BPC], f32)
            nc.vector.tensor_scalar_max(lensc_m[:], lensc_f[:], 1.0)
            invcnt = small.tile([128, JC, BPC], f32)
            nc.vector.reciprocal(out=invcnt[:], in_=lensc_m[:])

            # ---- softmax(mix_weights) * gamma ----
            mmax = small.tile([1, 1], f32)
            nc.vector.tensor_reduce(out=mmax[:], in_=mw_sb[:], axis=Axis.X, op=Alu.max)
            nmax = small.tile([1, 1], f32)
            nc.vector.tensor_scalar(out=nmax[:], in0=mmax[:], scalar1=-1.0, scalar2=None, op0=Alu.mult)
            mexp = small.tile([1, NL], f32)
            nc.scalar.activation(out=mexp[:], in_=mw_sb[:], func=Act.Exp, bias=nmax[:], scale=1.0)
            msum = small.tile([1, 1], f32)
            nc.vector.tensor_reduce(out=msum[:], in_=mexp[:], axis=Axis.X, op=Alu.add)
            mrec = small.tile([1, 1], f32)
            nc.vector.reciprocal(out=mrec[:], in_=msum[:])
            w_row = small.tile([1, NL], f32)
            nc.vector.tensor_scalar(out=w_row[:], in0=mexp[:], scalar1=mrec[:], scalar2=gam_sb[:], op0=Alu.mult, op1=Alu.mult)

            # ---- PE front block: all broadcast matmuls, then warm-up ----
            # w broadcast to [128, NL]
            ps_w = ps_o.tile([128, NL], f32, tag="po")
            nc.tensor.matmul(out=ps_w[:], lhsT=ones_f1[:], rhs=w_row[:], start=True, stop=True)
            w_sb = small.tile([128, NL], f32)
            nc.scalar.copy(w_sb[:], ps_w[:])
            if C2:
                # w as a column, then stacked per-partition w (w[p // 32])
                ps_wt = ps_b.tile([NL, 1], f32, tag="se", name="ps_wt")
                nc.tensor.matmul(out=ps_wt[:], lhsT=w_row[:], rhs=ones_f1[:, 0:1], start=True, stop=True)
                wt_sb = small.tile([NL, 1], f32)
                nc.scalar.copy(wt_sb[:], ps_wt[:])
                ps_ws = ps_b.tile([NL * C2, 1], f32, tag="se", name="ps_ws")
                nc.tensor.matmul(out=ps_ws[:], lhsT=sel32_sb[:], rhs=wt_sb[:], start=True, stop=True)
                wst_sb = small.tile([NL * C2, 1], f32)
                nc.scalar.copy(wst_sb[:], ps_ws[:])

            # starts/ends broadcast, packed two examples per SBUF tile so the
            # membership DVE ops below run at double width (half overhead)
            seps = []
            for g in range(BPC // 2):
                sep = sepool.tile([128, 2, 2, SL], f32, tag="sesb")
                for e in range(2):
                    b = 2 * g + e
                    ps_se = ps_b.tile([128, 2, SL], f32, tag="se")
                    sel_b = sel_sb[:, b * 128:(b + 1) * 128]
                    nc.tensor.matmul(out=ps_se[:, 0, :], lhsT=sel_b, rhs=starts_r[:], start=True, stop=True)
                    nc.tensor.matmul(out=ps_se[:, 1, :], lhsT=sel_b, rhs=ends_r[:], start=True, stop=True)
                    nc.scalar.copy(sep[:, e], ps_se[:])
                seps.append(sep)

            # p-state warm-up: keeps the PE clock ramped through the DMA fill
            warm = ps_o.tile([128, NOUT], f32, tag="po", name="warm")
            for _ in range(N_WARMUP):
                nc.tensor.matmul(out=warm[:], lhsT=ones_b[:, 0:128], rhs=ones_b[:, 0:NOUT], start=True, stop=True)

            # ---- per-pair membership (DVE, windowed, just-in-time; two
            # examples per op. Pool/GPSIMD lacks comparison opcodes so this
            # stays on DVE) ----
            def emit_memb(g):
                sep = seps[g]
                Mt = Mpool.tile([128, 2, PF, SL], bf16, tag="M")
                for c in range(PF):
                    j0 = 64 * c
                    w = SL - j0
                    csc = cs_sb[:, c:c + 1]
                    m2 = mpool.tile([128, 2, SL], f32, tag="m2")
                    nc.vector.tensor_scalar(
                        out=m2[:, :, :w], in0=sep[:, :, 1, j0:], scalar1=csc,
                        scalar2=None, op0=Alu.is_ge)
                    nc.vector.scalar_tensor_tensor(
                        out=Mt[:, :, c, j0:], in0=sep[:, :, 0, j0:], scalar=csc,
                        in1=m2[:, :, :w], op0=Alu.is_lt, op1=Alu.mult)
                Mc = None
                if C2:
                    # stacked tail membership with w folded in: rows (l, q)
                    j0 = 64 * PF
                    w = SL - j0
                    csc = cs_sb[:, PF:PF + 1]
                    Mc = Mpool.tile([128, 2, SL - 64 * PF], bf16, tag="Mc", name="Mc")
                    m2 = mpool.tile([128, 2, SL], f32, tag="m2")
                    nc.vector.tensor_scalar(
                        out=m2[:, :, :w], in0=sep[:, :, 1, j0:], scalar1=csc,
                        scalar2=wst_sb[:], op0=Alu.is_ge, op1=Alu.mult)
                    nc.vector.scalar_tensor_tensor(
                        out=Mc[:], in0=sep[:, :, 0, j0:], scalar=csc,
                        in1=m2[:, :, :w], op0=Alu.is_lt, op1=Alu.mult)
                return Mt, Mc

            # ---- per-example hidden loads (HWDGE sync FIFO) ----
            def emit_loads(b):
                ht = hpool.tile([128, NL, PF * H], bf16, tag="hm")
                ht2 = None
                if C2:
                    ht2 = hpool.tile([NL * C2, H], bf16, tag="hc", name="ht2")
                    nc.sync.dma_start(ht2[:], hidc[b, :, :])
                for l in range(NL):
                    nc.sync.dma_start(ht[:, l, :], hidm[b, :, l, :])
                return ht, ht2

            # ---- 4-layer running-sum mix in layer-arrival order (DVE).
            # The last add is split into chunk halves so the c0 pooling
            # matmuls can start while the c1 half is still mixing. ----
            def emit_mix(b, ht):
                mxm = mxpool.tile([128, PF * H], bf16, tag="mxm")
                tmp = mxpool.tile([128, PF * H], bf16, tag="tmp")
                nc.vector.tensor_scalar(out=mxm[:], in0=ht[:, 0, :], scalar1=w_sb[:, 0:1], scalar2=None, op0=Alu.mult)
                for l in range(1, NL):
                    nc.vector.tensor_scalar(out=tmp[:], in0=ht[:, l, :], scalar1=w_sb[:, l:l + 1], scalar2=None, op0=Alu.mult)
                    if l < NL - 1:
                        nc.vector.tensor_add(mxm[:], mxm[:], tmp[:])
                    else:
                        for c in range(PF):
                            s = slice(c * H, (c + 1) * H)
                            nc.vector.tensor_add(mxm[:, s], mxm[:, s], tmp[:, s])
                return mxm

            def emit_pool(b, mxm, ht2, Mt, Mc):
                # pooledT[h, j] = sum_p mixed[p, h] M[p, j]; the stacked tail
                # chunk contracts (layer, position) jointly with w in Mc.
                # c-outer order: all banks consume chunk c before chunk c+1,
                # so pooling starts as soon as the c0 half of the mix lands.
                e = b % 2
                ptsb = ptpool.tile([128, HC, SL], bf16, tag="pt")
                pps = [ps_p.tile([128, SL], f32, tag=f"pp{i}", name=f"pp{i}") for i in range(HC)]
                for c in range(PF):
                    j0 = 64 * c
                    for i in range(HC):
                        nc.tensor.matmul(
                            out=pps[i][:, j0:],
                            lhsT=mxm[:, c * H + i * 128: c * H + (i + 1) * 128],
                            rhs=Mt[:, e, c, j0:],
                            start=(c == 0),
                            stop=(c == PF - 1 and not C2),
                            skip_group_check=True,
                        )
                for i in range(HC):
                    if C2:
                        nc.tensor.matmul(
                            out=pps[i][:, 64 * PF:],
                            lhsT=ht2[:, i * 128:(i + 1) * 128],
                            rhs=Mc[:, e, :],
                            start=False,
                            stop=True,
                            skip_group_check=True,
                        )
                    nc.scalar.copy(ptsb[:, i, :], pps[i][:])
                return ptsb

            def emit_proj(b, ptsb):
                for jh in range(JC):
                    po = ps_o.tile([128, NOUT], f32, tag="po")
                    for i in range(HC):
                        nc.tensor.matmul(
                            out=po[:],
                            lhsT=ptsb[:, i, jh * 128:(jh + 1) * 128],
                            rhs=projT_sb[:, i, :],
                            start=(i == 0),
                            stop=(i == HC - 1),
                        )
                    osb = opool.tile([128, NOUT], f32, tag="o")
                    nc.scalar.activation(out=osb[:], in_=po[:], func=Act.Copy, scale=invcnt[:, jh, b:b + 1])
                    nc.scalar.dma_start(out[b, jh * 128:(jh + 1) * 128, :], osb[:])

            loads = [emit_loads(b) for b in range(BPC)]
            prev = None  # (b, ptsb) pending projection
            Mt = Mc = None
            for b in range(BPC):
                ht, ht2 = loads[b]
                if b % 2 == 0:
                    Mt, Mc = emit_memb(b // 2)
                mxm = emit_mix(b, ht)
                ptsb = emit_pool(b, mxm, ht2, Mt, Mc)
                if prev is not None:
                    emit_proj(*prev)
                prev = (b, ptsb)
            emit_proj(*prev)

    nc.finalize()
    return nc


def _get_nc(pmax):
    if pmax not in _NC_CACHE:
        _NC_CACHE[pmax] = _build_nc(pmax)
    return _NC_CACHE[pmax]


def kernel(subwords=None, bert_lens=None, bert_mask=None, hidden_states=None,
           mix_weights=None, gamma=None, proj_w=None, **_ignored):
    global LAST_RESULT
    import os
    import ml_dtypes
    from concourse.bass_utils import run_bass_kernel_spmd

    bf16 = ml_dtypes.bfloat16
    lens_np = np.asarray(bert_lens).astype(np.int32)

    # pick the smallest compiled position-prefix that covers every example
    need = int(lens_np.sum(axis=1).max())
    pmax = PMAX_DEFAULT
    if need > pmax:
        pmax = 384 if need <= 384 else 512
    nc = _get_nc(pmax)
    PF = pmax // 128
    C2 = pmax - 128 * PF
    CH = PF + (1 if C2 else 0)

    hs = np.asarray(hidden_states, dtype=np.float32).astype(bf16)  # [NL,B,SW,H]
    mw_np = np.asarray(mix_weights, dtype=np.float32).reshape(1, NL)
    gam_np = np.asarray(gamma, dtype=np.float32).reshape(1, 1)
    # projT[p, i*NOUT + o] = proj_w[o, 128*i + p]
    projT_np = np.ascontiguousarray(
        np.asarray(proj_w, dtype=np.float32).astype(bf16).T
        .reshape(HC, 128, NOUT).transpose(1, 0, 2).reshape(128, HC * NOUT))
    sel_np = np.zeros((BPC, BPC * 128), dtype=np.float32)
    for b in range(BPC):
        sel_np[b, b * 128:(b + 1) * 128] = 1.0
    cs_np = np.empty((128, CH), dtype=np.float32)
    p = np.arange(128)
    for c in range(PF):
        cs_np[:, c] = 1 + 128 * c + p
    if C2:
        cs_np[:, PF] = 1 + 128 * PF + (p % C2)
        sel32_np = np.zeros((NL, NL * C2), dtype=np.float32)
        for l in range(NL):
            sel32_np[l, l * C2:(l + 1) * C2] = 1.0

    in_maps = []
    for cidx in range(NCORES):
        sl = slice(cidx * BPC, (cidx + 1) * BPC)
        hsb = hs[:, sl]  # [NL, BPC, SW, H]
        # hidm[b, part, l, c*H + h] = hs[l, b, 128c + part, h]
        hidm = np.ascontiguousarray(
            hsb[:, :, :128 * PF].reshape(NL, BPC, PF, 128, H)
            .transpose(1, 3, 0, 2, 4).reshape(BPC, 128, NL, PF * H))
        m = {
            "hidm": hidm,
            "lens": np.ascontiguousarray(lens_np[sl]),
            "mw": mw_np,
            "gam": gam_np,
            "projT": projT_np,
            "sel": sel_np,
            "csall": cs_np,
        }
        if C2:
            # stacked tail: partition 32l + q = layer l, position 128*PF + q
            m["hidc"] = np.ascontiguousarray(
                hsb[:, :, 128 * PF:128 * PF + C2].transpose(1, 0, 2, 3)
                .reshape(BPC, NL * C2, H))
            m["sel32"] = sel32_np
        in_maps.append(m)

    trace = bool(int(os.environ.get("KERNEL_TRACE", "0")))
    LAST_RESULT = run_bass_kernel_spmd(nc, in_maps, list(range(NCORES)), trace=trace)
    res = LAST_RESULT.results
    return np.concatenate([np.asarray(r["out"], dtype=np.float32) for r in res], axis=0)
